# revision 1
# baseline (speedup 1.0000x reference)
"""Multi-head causal self-attention (D=768, H=12, S=4096) on 8 Trainium2 cores.

Sharding: 4 head-groups (3 heads each) x 2 interleaved query-sets.
Core c = 2*g + s owns head-group g (heads 3g..3g+2) and query 128-row
blocks s, s+2, s+4, ... (even/odd interleave balances the causal
triangle).  Every core runs the SAME program; per-core behaviour is
driven entirely by input data (weight slices, gathered query columns,
additive causal masks).  Each core produces a partial [2048, 768]
output (its heads pushed through its slice of Wo, all biases folded
in); the host sums the 4 group partials per query-set and re-interleaves
rows.

Layout notes (per core):
  - scores are computed transposed: S_T[k, q] = K_h Q_h^T so the PV
    matmul needs no transposes; the softmax denominator comes from a
    ones-column appended to V.
  - softmax skips max-subtraction (scores are O(1) by construction).
  - causality: a compile-time query-suffix trim [q0:512] valid for both
    parities (QK / exp / PV all sliced), plus per-core multiplicative 0/1
    bf16 mask data applied to the probabilities on the boundary k-blocks.
  - projection chunk qg (Q/K/V for the blocks group qg introduces) is
    emitted just before attention group qg so exp overlaps later chunks.
"""

import numpy as np

D = 768
S = 4096
H = 12
HD = 64
NG = 4          # head groups
GH = 3          # heads per group
GD = GH * HD    # 192 dims per group
SL = S // 2     # local queries per core (2048)
P = 128
NC = D // P     # 6 contraction chunks
QG = 4          # query groups per core (512 q each)
QGS = 512
NKB = S // P    # 32 key blocks
NQB = SL // P   # 16 local query tiles
MASKVAL = -30000.0

_CACHE = {}


def _build_program():
    import concourse.bacc as bacc
    import concourse.mybir as mybir
    import concourse.tile as tile
    from contextlib import ExitStack

    bf16 = mybir.dt.bfloat16
    f32 = mybir.dt.float32
    f32r = mybir.dt.float32r

    nc = bacc.Bacc("TRN2", target_bir_lowering=False, debug=False, num_devices=8)

    xt = nc.dram_tensor("xt", [D, S], bf16, kind="ExternalInput").ap()
    xtq = nc.dram_tensor("xtq", [D, SL], bf16, kind="ExternalInput").ap()
    wqt = nc.dram_tensor("wqt", [D, GD], bf16, kind="ExternalInput").ap()
    wkt = nc.dram_tensor("wkt", [D, GD], bf16, kind="ExternalInput").ap()
    wvt = nc.dram_tensor("wvt", [D, GD], bf16, kind="ExternalInput").ap()
    wota0 = nc.dram_tensor("wota0", [P, D], bf16, kind="ExternalInput").ap()
    wota1 = nc.dram_tensor("wota1", [65, D], bf16, kind="ExternalInput").ap()
    bias = nc.dram_tensor("bias", [P, 4], f32, kind="ExternalInput").ap()
    masks = nc.dram_tensor("masks", [P, 8 * QGS], bf16, kind="ExternalInput").ap()
    out = nc.dram_tensor("out", [SL, D], f32, kind="ExternalOutput").ap()

    Exp = mybir.ActivationFunctionType.Exp
    mult = mybir.AluOpType.mult
    add = mybir.AluOpType.add

    with tile.TileContext(nc) as tc, ExitStack() as ctx:
        const = ctx.enter_context(tc.tile_pool(name="const", bufs=1))

        # ---- persistent SBUF tiles ----
        xt_sb = const.tile([P, NC, S], bf16, tag="xt")
        xtq_sb = const.tile([P, NC, SL], bf16, tag="xtq")
        wqt_sb = const.tile([P, NC, GD], bf16, tag="wqt")
        wkt_sb = const.tile([P, NC, GD], bf16, tag="wkt")
        wvt_sb = const.tile([P, NC, GD], bf16, tag="wvt")
        wota0_sb = const.tile([P, D], bf16, tag="wota0")
        wota1_sb = const.tile([65, D], bf16, tag="wota1")
        bias_sb = const.tile([P, 4], f32, tag="bias")
        mask_sb = const.tile([P, 8 * QGS], bf16, tag="masks")
        kt01_sb = const.tile([P, S], bf16, tag="kt01")    # heads 0,1 stacked (64+64)
        kt2_sb = const.tile([64, S], bf16, tag="kt2")
        qt01_sb = const.tile([P, SL], bf16, tag="qt01")
        qt2_sb = const.tile([64, SL], bf16, tag="qt2")
        # V per head: [128 k-part, kb, 65] with col 64 = 1.0 (denominator)
        v_sb = [const.tile([P, NKB, 65], bf16, tag=f"v{h}", name=f"v{h}")
                for h in range(GH)]
        ones_sb = const.tile([1, 64], bf16, tag="ones")

        # ---- load constants / inputs ----
        xt_r = xt.rearrange("(c p) s -> c p s", p=P)
        xtq_r = xtq.rearrange("(c p) s -> c p s", p=P)
        wqt_r = wqt.rearrange("(c p) d -> c p d", p=P)
        wkt_r = wkt.rearrange("(c p) d -> c p d", p=P)
        wvt_r = wvt.rearrange("(c p) d -> c p d", p=P)
        for c in range(NC):
            nc.sync.dma_start(wqt_sb[:, c, :], wqt_r[c])
            nc.sync.dma_start(xtq_sb[:, c, :], xtq_r[c])
            nc.sync.dma_start(wkt_sb[:, c, :], wkt_r[c])
            nc.sync.dma_start(wvt_sb[:, c, :], wvt_r[c])
            nc.sync.dma_start(xt_sb[:, c, :], xt_r[c])
        nc.sync.dma_start(wota0_sb[:], wota0[:])
        nc.sync.dma_start(wota1_sb[:], wota1[:])
        nc.sync.dma_start(bias_sb[:], bias[:])
        nc.sync.dma_start(mask_sb[:], masks[:])
        nc.vector.memset(ones_sb[:], 1.0)
        for h in range(GH):
            # whole tile -> 1.0; V evicts overwrite cols 0..63 of each block,
            # leaving col 64 as the denominator ones-column
            nc.vector.memset(v_sb[h][:], 1.0)

        # ---- interleaved projection chunks + attention groups ----
        # PSUM budget (8 banks): stA tag 2x[128,1024]=4, stB tag 1x[128,512]=1,
        # pv tag 3x[65,512]=3.  Projection psums + out-proj + bcast reuse the
        # same tags so attention group g can overlap projection chunk g+1.
        kq = [(kt01_sb, 0), (kt01_sb, 64), (kt2_sb, 0)]  # (tile, base partition)
        qq = [(qt01_sb, 0), (qt01_sb, 64), (qt2_sb, 0)]

        with tc.tile_pool(name="stA_ps", bufs=2, space="PSUM") as stAps, \
             tc.tile_pool(name="stB_ps", bufs=1, space="PSUM") as stBps, \
             tc.tile_pool(name="pv_ps", bufs=3, space="PSUM") as pvps, \
             tc.tile_pool(name="pt", bufs=8) as ptpool, \
             tc.tile_pool(name="stk", bufs=2) as stkpool, \
             tc.tile_pool(name="nrm", bufs=3) as nrmpool, \
             tc.tile_pool(name="oev", bufs=4) as oevpool:
            for qg in range(QG):
                # -- projection chunk qg: Q group qg, K blocks 8qg..8qg+7,
                #    V blocks 8qg..8qg+7 --
                ps = stAps.tile([P, QGS], f32, tag="stA", name=f"qps{qg}")
                for c in range(NC):
                    nc.tensor.matmul(
                        ps[:], wqt_sb[:, c, 0:128],
                        xtq_sb[:, c, qg * QGS:(qg + 1) * QGS],
                        start=(c == 0), stop=(c == NC - 1))
                nc.vector.tensor_scalar(
                    qt01_sb[:, qg * QGS:(qg + 1) * QGS], ps[:],
                    0.125, bias_sb[:, 0:1], mult, add)
                ps2 = stBps.tile([64, QGS], f32, tag="stB", name=f"qps2_{qg}")
                for c in range(NC):
                    nc.tensor.matmul(
                        ps2[:], wqt_sb[:, c, 128:192],
                        xtq_sb[:, c, qg * QGS:(qg + 1) * QGS],
                        start=(c == 0), stop=(c == NC - 1))
                nc.vector.tensor_scalar(
                    qt2_sb[:, qg * QGS:(qg + 1) * QGS], ps2[:],
                    0.125, bias_sb[0:64, 1:2], mult, add)
                for kg in (2 * qg, 2 * qg + 1):
                    ps = stAps.tile([P, QGS], f32, tag="stA", name=f"kps{kg}")
                    for c in range(NC):
                        nc.tensor.matmul(
                            ps[:], wkt_sb[:, c, 0:128],
                            xt_sb[:, c, kg * QGS:(kg + 1) * QGS],
                            start=(c == 0), stop=(c == NC - 1))
                    nc.vector.tensor_scalar(
                        kt01_sb[:, kg * QGS:(kg + 1) * QGS], ps[:],
                        bias_sb[:, 2:3], None, add)
                    ps2 = stBps.tile([64, QGS], f32, tag="stB", name=f"kps2_{kg}")
                    for c in range(NC):
                        nc.tensor.matmul(
                            ps2[:], wkt_sb[:, c, 128:192],
                            xt_sb[:, c, kg * QGS:(kg + 1) * QGS],
                            start=(c == 0), stop=(c == NC - 1))
                    nc.vector.tensor_scalar(
                        kt2_sb[:, kg * QGS:(kg + 1) * QGS], ps2[:],
                        bias_sb[0:64, 3:4], None, add)
                for kb in range(8 * qg, 8 * qg + 8):
                    psv = pvps.tile([P, GD], f32, tag="pv", name=f"vps{kb}")
                    for c in range(NC):
                        nc.tensor.matmul(
                            psv[:], xt_sb[:, c, kb * P:(kb + 1) * P], wvt_sb[:, c, :],
                            start=(c == 0), stop=(c == NC - 1))
                    for h in range(GH):
                        nc.vector.tensor_copy(
                            v_sb[h][:, kb, 0:64], psv[:, h * HD:(h + 1) * HD])

                # -- attention group qg --
                kcnt = 8 * (qg + 1)
                pv = [pvps.tile([65, QGS], f32, tag="pv", name=f"pv{qg}_{h}")
                      for h in range(GH)]
                for m in range(kcnt):
                    # causal query-suffix trim: for key block m, local query
                    # tiles j < ceil((m-s)/2) - 4qg are entirely below the
                    # diagonal for BOTH parities when using s=1's bound
                    # ceil((m-1)/2); parity-dependent leftovers are handled by
                    # the data mask inside the remaining span.
                    q0b = max(0, -(-(m - 1) // 2) - 4 * qg) if m > 0 else 0
                    q0 = P * q0b
                    vspan = QGS - q0
                    stA = stAps.tile([P, 2 * QGS], f32, tag="stA", name=f"stA{qg}_{m}")
                    stB = stBps.tile([P, QGS], f32, tag="stB", name=f"stB{qg}_{m}")
                    for h in range(GH):
                        kt_t, kb_p = kq[h]
                        qt_t, qb_p = qq[h]
                        dst = (stA[:, h * QGS + q0:(h + 1) * QGS] if h < 2
                               else stB[:, q0:])
                        nc.tensor.matmul(
                            dst,
                            kt_t[kb_p:kb_p + 64, m * P:(m + 1) * P],
                            qt_t[qb_p:qb_p + 64, qg * QGS + q0:(qg + 1) * QGS],
                            start=True, stop=True)
                    ptA = ptpool.tile([P, 2 * QGS], bf16, tag="ptA")
                    ptB = ptpool.tile([P, QGS], bf16, tag="ptB", bufs=12)
                    stA_v = stA[:].rearrange("p (h x) -> p h x", x=QGS)
                    ptA_v = ptA[:].rearrange("p (h x) -> p h x", x=QGS)
                    nc.scalar.activation(ptA_v[:, :, q0:], stA_v[:, :, q0:], Exp)
                    nc.scalar.activation(ptB[:, q0:], stB[:, q0:], Exp)
                    if m >= 8 * qg:
                        # multiplicative 0/1 causal mask on the probabilities
                        w = m - 8 * qg
                        span = P * (w // 2 + 1)
                        if span > q0:
                            for h in range(GH):
                                dst = (ptA[:, h * QGS + q0:h * QGS + span] if h < 2
                                       else ptB[:, q0:span])
                                nc.vector.tensor_tensor(
                                    dst, dst,
                                    mask_sb[:, w * QGS + q0:w * QGS + span], mult)
                    for h in range(GH):
                        src = (ptA[:, h * QGS + q0:(h + 1) * QGS] if h < 2
                               else ptB[:, q0:])
                        nc.tensor.matmul(
                            pv[h][:, q0:], v_sb[h][:, m, :], src,
                            start=(m == 0), stop=(m == kcnt - 1))
                # normalize: out_h = pv_h[0:64] / pv_h[64] ; stack for out-proj
                stk0 = stkpool.tile([P, QGS], bf16, tag="sc0")
                stk1 = stkpool.tile([65, QGS], bf16, tag="sc1")
                nc.vector.memset(stk1[64:65, :], 1.0)
                for h in range(GH):
                    recip = nrmpool.tile([1, QGS], bf16, tag="recip")
                    with nc.allow_low_precision(
                            reason="bf16 softmax denominators, ~0.4% rel"):
                        nc.vector.reciprocal(recip[:], pv[h][64:65, :])
                    bcast = stBps.tile([64, QGS], f32, tag="stB", name=f"bc{qg}_{h}")
                    nc.tensor.matmul(
                        bcast[:], ones_sb[:], recip[:], start=True, stop=True)
                    tmp = nrmpool.tile([64, QGS], bf16, tag="tmp")
                    nc.vector.tensor_copy(tmp[:], pv[h][0:64, :])
                    tgt = stk0[h * 64:(h + 1) * 64, :] if h < 2 else stk1[0:64, :]
                    nc.vector.tensor_tensor(tgt, tmp[:], bcast[:], mult)
                # output projection for this group's 4 query tiles
                for jj in range(QG):
                    op = stAps.tile([P, 1024], f32, tag="stA", name=f"op{qg}_{jj}")
                    for half in range(2):
                        nc.tensor.matmul(
                            op[:, half * 512:half * 512 + 384],
                            stk0[:, jj * P:(jj + 1) * P],
                            wota0_sb[:, half * 384:(half + 1) * 384],
                            start=True, stop=False)
                        nc.tensor.matmul(
                            op[:, half * 512:half * 512 + 384],
                            stk1[:, jj * P:(jj + 1) * P],
                            wota1_sb[:, half * 384:(half + 1) * 384],
                            start=False, stop=True)
                    oe = oevpool.tile([P, D], f32, tag="oe")
                    opv = op[:].rearrange("p (t x) -> p t x", x=512)
                    nc.vector.tensor_copy(
                        oe[:].rearrange("p (t x) -> p t x", x=384), opv[:, :, 0:384])
                    jq = 4 * qg + jj
                    nc.sync.dma_start(out[jq * P:(jq + 1) * P, :], oe[:])

    nc.compile()
    return nc


def _host_prep(inputs, Wq, bq, Wk, bk, Wv, bv, Wo, bo):
    import ml_dtypes

    bf16 = ml_dtypes.bfloat16
    X = np.asarray(inputs, np.float32).reshape(S, D)
    XT = np.ascontiguousarray(X.T)                      # [768, 4096]
    XT_bf = XT.astype(bf16)
    # query-set gathers: blocks s, s+2, ... of 32 128-col blocks
    XTb = XT.reshape(D, NKB // 2, 2, P)
    XTq = [np.ascontiguousarray(XTb[:, :, s, :].reshape(D, SL)).astype(bf16)
           for s in range(2)]

    # per-core multiplicative causal masks [128, 8*512], 1=keep 0=drop
    # (S_T layout: k on partitions, q on free dim)
    tri = (np.arange(P)[None, :] >= np.arange(P)[:, None]).astype(np.float32)
    mk = []
    for s_ in range(2):
        m = np.ones((P, 8, QGS), np.float32)
        for w in range(8):
            npref = max(0, -(-(w - s_) // 2))  # ceil((w - s)/2) clamped at 0
            m[:, w, :P * npref] = 0.0
            if w >= s_ and (w - s_) % 2 == 0:
                dblk = (w - s_) // 2
                m[:, w, dblk * P:(dblk + 1) * P] = tri
        mk.append(np.ascontiguousarray(m.reshape(P, 8 * QGS)).astype(bf16))

    in_maps = []
    for g in range(NG):
        hs = slice(GD * g, GD * (g + 1))
        WqT = np.ascontiguousarray(Wq[hs, :].T).astype(bf16)
        WkT = np.ascontiguousarray(Wk[hs, :].T).astype(bf16)
        WvT = np.ascontiguousarray(Wv[hs, :].T).astype(bf16)
        WoT = np.ascontiguousarray(Wo[:, hs].T).astype(np.float32)  # [192, 768]
        bo_g = bv[hs].astype(np.float32) @ WoT
        if g == 0:
            bo_g = bo_g + bo.astype(np.float32)
        wota = np.concatenate([WoT, bo_g[None, :]], axis=0)  # [193, 768]
        wota0 = np.ascontiguousarray(wota[0:P]).astype(bf16)
        wota1 = np.ascontiguousarray(wota[P:]).astype(bf16)
        bias_t = np.zeros((P, 4), np.float32)
        bias_t[:, 0] = bq[hs][0:128] / 8.0
        bias_t[0:64, 1] = bq[hs][128:192] / 8.0
        bias_t[:, 2] = bk[hs][0:128]
        bias_t[0:64, 3] = bk[hs][128:192]
        for s_ in range(2):
            in_maps.append({
                "xt": XT_bf, "xtq": XTq[s_],
                "wqt": WqT, "wkt": WkT, "wvt": WvT,
                "wota0": wota0, "wota1": wota1,
                "bias": bias_t, "masks": mk[s_],
            })
    return in_maps


def _gather(results):
    out = np.zeros((S, D), np.float32)
    ov = out.reshape(NQB, 2, P, D)
    for s_ in range(2):
        acc = np.zeros((SL, D), np.float32)
        for g in range(NG):
            acc += np.asarray(results[2 * g + s_]["out"], np.float32)
        ov[:, s_, :, :] = acc.reshape(NQB, P, D)
    return out.reshape(1, S, D)


def kernel(inputs, Wq, bq, Wk, bk, Wv, bv, Wo, bo):
    from concourse.bass_utils import run_bass_kernel_spmd

    if "nc" not in _CACHE:
        _CACHE["nc"] = _build_program()
    nc = _CACHE["nc"]
    in_maps = _host_prep(
        np.asarray(inputs), np.asarray(Wq), np.asarray(bq), np.asarray(Wk),
        np.asarray(bk), np.asarray(Wv), np.asarray(bv), np.asarray(Wo),
        np.asarray(bo))
    # core order: core = 2*g + s, but in_maps was built g-major with s inner,
    # i.e. in_maps[2*g + s] already matches core id 2*g + s.
    res = run_bass_kernel_spmd(nc, in_maps, list(range(8))).results
    return _gather(res)



# revision 3
# speedup vs baseline: 1.0500x; 1.0500x over previous
"""Multi-head causal self-attention (D=768, H=12, S=4096) on 8 Trainium2 cores.

Sharding: 4 head-groups (3 heads each) x 2 interleaved query-sets.
Core c = 2*g + s owns head-group g (heads 3g..3g+2) and query 128-row
blocks s, s+2, s+4, ... (even/odd interleave balances the causal
triangle).  Every core runs the SAME program; per-core behaviour is
driven entirely by input data.  Each core produces a partial [2048, 768]
output (its heads pushed through its slice of Wo, all biases folded
in); the host sums the 4 group partials per query-set and re-interleaves
rows.

v2 vs baseline:
  - QK^T matmuls run as fp8e4 DoubleRow (2x fewer PE columns): Q and K
    are stored fp8 in zero-padded [128, 2, *] tiles so all three heads
    share one stationary operand; zero regions kill cross-head terms.
  - Q/K projections run as fp8 DoubleRow with dual-rail (hi+lo) fp8
    weights (weights exact to ~0.1%; activations single-fp8).
  - scores stay unscaled until exp (scale=1/8 folded into the Exp).
  - causal-mask multiplies moved to the idle GPSIMD engine.
  - V eviction merged to one instr per key block; V/P/O paths stay bf16
    (fp8 there costs ~2.4% output noise, over budget).
  - input DMAs are windowed per query-group and JIT-ordered; weights /
    zero-fills ride the SWDGE (gpsimd) queue.
"""

import numpy as np

D = 768
S = 4096
H = 12
HD = 64
NG = 4          # head groups
GH = 3          # heads per group
GD = GH * HD    # 192 dims per group
SL = S // 2     # local queries per core (2048)
P = 128
NC = D // P     # 6 contraction chunks
QG = 4          # query groups per core (512 q each)
QGS = 512
NKB = S // P    # 32 key blocks
NQB = SL // P   # 16 local query tiles

DRPROJ = True   # Q/K projections via fp8 dual-rail DoubleRow
WSCALE = 16.0   # pow-2 scale keeping dual-rail fp8 weights in normal range

_CACHE = {}


def _build_program():
    import concourse.bacc as bacc
    import concourse.mybir as mybir
    import concourse.tile as tile
    from contextlib import ExitStack

    bf16 = mybir.dt.bfloat16
    f8 = mybir.dt.float8e4
    f32 = mybir.dt.float32

    nc = bacc.Bacc("TRN2", target_bir_lowering=False, debug=False, num_devices=8)

    xt = nc.dram_tensor("xt", [D, S], bf16, kind="ExternalInput").ap()
    bias = nc.dram_tensor("bias", [P, 4], f32, kind="ExternalInput").ap()
    masks = nc.dram_tensor("masks", [P, 8 * QGS], bf16, kind="ExternalInput").ap()
    wvt = nc.dram_tensor("wvt", [D, GD], bf16, kind="ExternalInput").ap()
    wota0 = nc.dram_tensor("wota0", [P, D], bf16, kind="ExternalInput").ap()
    wota1 = nc.dram_tensor("wota1", [65, D], bf16, kind="ExternalInput").ap()
    zeros8 = nc.dram_tensor("zeros8", [P, 2 * SL], f8, kind="ExternalInput").ap()
    out = nc.dram_tensor("out", [SL, D], f32, kind="ExternalOutput").ap()
    if DRPROJ:
        xt8 = nc.dram_tensor("xt8", [D, S], f8, kind="ExternalInput").ap()
        xtq8 = nc.dram_tensor("xtq8", [D, SL], f8, kind="ExternalInput").ap()
        wq2 = nc.dram_tensor("wq2", [P, NC, 2, GD], f8, kind="ExternalInput").ap()
        wk2 = nc.dram_tensor("wk2", [P, NC, 2, GD], f8, kind="ExternalInput").ap()
    else:
        xtq = nc.dram_tensor("xtq", [D, SL], bf16, kind="ExternalInput").ap()
        wqt = nc.dram_tensor("wqt", [D, GD], bf16, kind="ExternalInput").ap()
        wkt = nc.dram_tensor("wkt", [D, GD], bf16, kind="ExternalInput").ap()

    Exp = mybir.ActivationFunctionType.Exp
    mult = mybir.AluOpType.mult
    add = mybir.AluOpType.add
    DR = mybir.MatmulPerfMode.DoubleRow

    with tile.TileContext(nc) as tc, ExitStack() as ctx:
        const = ctx.enter_context(tc.tile_pool(name="const", bufs=1))

        # ---- persistent SBUF tiles ----
        xt_sb = const.tile([P, NC, S], bf16, tag="xt")
        wvt_sb = const.tile([P, NC, GD], bf16, tag="wvt")
        wota0_sb = const.tile([P, D], bf16, tag="wota0")
        wota1_sb = const.tile([65, D], bf16, tag="wota1")
        bias_sb = const.tile([P, 4], f32, tag="bias")
        mask_sb = const.tile([P, 8 * QGS], bf16, tag="masks")
        if DRPROJ:
            xt8_sb = const.tile([P, NC, S], f8, tag="xt8")
            xtq8_sb = const.tile([P, NC, SL], f8, tag="xtq8")
            wq2_sb = const.tile([P, NC, 2, GD], f8, tag="wq2")
            wk2_sb = const.tile([P, NC, 2, GD], f8, tag="wk2")
        else:
            xtq_sb = const.tile([P, NC, SL], bf16, tag="xtq")
            wqt_sb = const.tile([P, NC, GD], bf16, tag="wqt")
            wkt_sb = const.tile([P, NC, GD], bf16, tag="wkt")
        # fp8 K cache, 3 heads packed into (partition-half, rail):
        #   (p 0:64,  rail 0) = head0 dims, (p 64:128, rail 0) = head1 dims,
        #   (p 0:64,  rail 1) = head2 dims, (p 64:128, rail 1) = ZERO.
        k8_sb = const.tile([P, 2, S], f8, tag="k8")
        # per-head fp8 Q, same packing; zeros everywhere except own region so
        # the shared K stationary operand only contracts the own head's dims.
        q8_sb = [const.tile([P, 2, SL], f8, tag=f"q8{h}", name=f"q8{h}")
                 for h in range(GH)]
        # V per key block: [128 k-part, kb, head, 65] with col 64 = 1.0
        v_sb = const.tile([P, NKB, GH, 65], bf16, tag="vall")
        ones_sb = const.tile([1, 64], bf16, tag="ones")

        # ---- DMAs: weights/zeros on the gpsimd (SWDGE) queue ----
        if DRPROJ:
            nc.gpsimd.dma_start(wq2_sb[:], wq2[:])
            nc.gpsimd.dma_start(wk2_sb[:], wk2[:])
        nc.gpsimd.dma_start(bias_sb[:], bias[:])
        for h in range(GH):
            nc.gpsimd.dma_start(q8_sb[h][:], zeros8.rearrange("p (a b) -> p a b", a=2))
        nc.gpsimd.dma_start(k8_sb[64:128, 1, :], zeros8[0:64, :])
        nc.gpsimd.dma_start(mask_sb[:], masks[:])
        nc.gpsimd.dma_start(wvt_sb[:], wvt.rearrange("(c p) d -> p c d", p=P))
        nc.gpsimd.dma_start(wota0_sb[:], wota0[:])
        nc.gpsimd.dma_start(wota1_sb[:], wota1[:])

        # ---- input windows, JIT-ordered per query group (SP queue) ----
        xt_r = xt.rearrange("(c p) s -> p c s", p=P)
        if DRPROJ:
            xt8_r = xt8.rearrange("(c p) s -> p c s", p=P)
            xtq8_r = xtq8.rearrange("(c p) s -> p c s", p=P)
        else:
            xtq_r = xtq.rearrange("(c p) s -> p c s", p=P)
        for qg in range(QG):
            qw = slice(qg * QGS, (qg + 1) * QGS)
            kw = slice(2 * qg * QGS, (2 * qg + 2) * QGS)
            if DRPROJ:
                nc.sync.dma_start(xtq8_sb[:, :, qw], xtq8_r[:, :, qw])
                nc.sync.dma_start(xt8_sb[:, :, kw], xt8_r[:, :, kw])
            else:
                nc.sync.dma_start(xtq_sb[:, :, qw], xtq_r[:, :, qw])
            nc.sync.dma_start(xt_sb[:, :, kw], xt_r[:, :, kw])

        nc.vector.memset(ones_sb[:], 1.0)
        nc.vector.memset(v_sb[:, :, :, 64:65], 1.0)

        evsc = 1.0 / WSCALE if DRPROJ else 1.0

        def proj_qk(ps, w2_sb, wt_sb, x8_sb, x_sb, cols, mrange):
            """contraction loop for one Q/K projection psum"""
            n = cols.stop - cols.start
            for c in range(NC):
                if DRPROJ:
                    rhs = (x8_sb[:, c, cols].unsqueeze(1)
                           .broadcast_to([P, 2, n]))
                    nc.tensor.matmul(ps[:], w2_sb[:, c, :, mrange], rhs,
                                     start=(c == 0), stop=(c == NC - 1),
                                     perf_mode=DR)
                else:
                    nc.tensor.matmul(ps[:], wt_sb[:, c, mrange],
                                     x_sb[:, c, cols],
                                     start=(c == 0), stop=(c == NC - 1))

        with tc.tile_pool(name="stA_ps", bufs=2, space="PSUM") as stAps, \
             tc.tile_pool(name="stB_ps", bufs=1, space="PSUM") as stBps, \
             tc.tile_pool(name="pv_ps", bufs=3, space="PSUM") as pvps, \
             tc.tile_pool(name="pt", bufs=8) as ptpool, \
             tc.tile_pool(name="stk", bufs=2) as stkpool, \
             tc.tile_pool(name="nrm", bufs=3) as nrmpool, \
             tc.tile_pool(name="oev", bufs=4) as oevpool:
            for qg in range(QG):
                qw = slice(qg * QGS, (qg + 1) * QGS)
                # -- Q projection for group qg --
                ps = stAps.tile([P, QGS], f32, tag="stA", name=f"qps{qg}")
                proj_qk(ps, DRPROJ and wq2_sb, not DRPROJ and wqt_sb,
                        DRPROJ and xtq8_sb, not DRPROJ and xtq_sb,
                        qw, slice(0, 128))
                nc.vector.tensor_scalar(
                    q8_sb[0][0:64, 0, qw], ps[0:64, :],
                    evsc, bias_sb[0:64, 0:1], mult, add)
                nc.vector.tensor_scalar(
                    q8_sb[1][64:128, 0, qw], ps[64:128, :],
                    evsc, bias_sb[64:128, 0:1], mult, add)
                ps2 = stBps.tile([64, QGS], f32, tag="stB", name=f"qps2_{qg}")
                proj_qk(ps2, DRPROJ and wq2_sb, not DRPROJ and wqt_sb,
                        DRPROJ and xtq8_sb, not DRPROJ and xtq_sb,
                        qw, slice(128, 192))
                nc.vector.tensor_scalar(
                    q8_sb[2][0:64, 1, qw], ps2[:],
                    evsc, bias_sb[0:64, 1:2], mult, add)
                # -- K projection for key groups 2qg, 2qg+1 --
                for kg in (2 * qg, 2 * qg + 1):
                    kcols = slice(kg * QGS, (kg + 1) * QGS)
                    ps = stAps.tile([P, QGS], f32, tag="stA", name=f"kps{kg}")
                    proj_qk(ps, DRPROJ and wk2_sb, not DRPROJ and wkt_sb,
                            DRPROJ and xt8_sb, not DRPROJ and xt_sb,
                            kcols, slice(0, 128))
                    nc.vector.tensor_scalar(
                        k8_sb[:, 0, kcols], ps[:],
                        evsc, bias_sb[:, 2:3], mult, add)
                    ps2 = stBps.tile([64, QGS], f32, tag="stB", name=f"kps2_{kg}")
                    proj_qk(ps2, DRPROJ and wk2_sb, not DRPROJ and wkt_sb,
                            DRPROJ and xt8_sb, not DRPROJ and xt_sb,
                            kcols, slice(128, 192))
                    nc.vector.tensor_scalar(
                        k8_sb[0:64, 1, kcols], ps2[:],
                        evsc, bias_sb[0:64, 3:4], mult, add)
                # -- V projection for key blocks 8qg..8qg+7 --
                for kb in range(8 * qg, 8 * qg + 8):
                    psv = pvps.tile([P, GD], f32, tag="pv", name=f"vps{kb}")
                    for c in range(NC):
                        nc.tensor.matmul(
                            psv[:], xt_sb[:, c, kb * P:(kb + 1) * P],
                            wvt_sb[:, c, :],
                            start=(c == 0), stop=(c == NC - 1))
                    nc.vector.tensor_copy(
                        v_sb[:, kb, :, 0:64],
                        psv[:].rearrange("p (h d) -> p h d", d=HD))

                # -- attention group qg --
                kcnt = 8 * (qg + 1)
                pv = [pvps.tile([65, QGS], f32, tag="pv", name=f"pv{qg}_{h}")
                      for h in range(GH)]
                for m in range(kcnt):
                    # causal query-suffix trim (see baseline): key block m is
                    # entirely below the causal diagonal for local query tiles
                    # j < ceil((m-1)/2) - 4qg under both parities; the data
                    # mask handles the per-parity leftovers.
                    q0 = P * (max(0, -(-(m - 1) // 2) - 4 * qg) if m > 0 else 0)
                    stA = stAps.tile([P, 2 * QGS], f32, tag="stA",
                                     name=f"stA{qg}_{m}")
                    stB = stBps.tile([P, QGS], f32, tag="stB",
                                     name=f"stB{qg}_{m}")
                    kblk = k8_sb[:, :, m * P:(m + 1) * P]
                    for h in range(GH):
                        dst = (stA[:, h * QGS + q0:(h + 1) * QGS] if h < 2
                               else stB[:, q0:])
                        nc.tensor.matmul(
                            dst, kblk,
                            q8_sb[h][:, :, qg * QGS + q0:(qg + 1) * QGS],
                            start=True, stop=True, perf_mode=DR)
                    ptA = ptpool.tile([P, 2 * QGS], bf16, tag="ptA")
                    ptB = ptpool.tile([P, QGS], bf16, tag="ptB", bufs=12)
                    stA_v = stA[:].rearrange("p (h x) -> p h x", x=QGS)
                    ptA_v = ptA[:].rearrange("p (h x) -> p h x", x=QGS)
                    # scores are raw q.k; the 1/sqrt(64) lives in the Exp scale
                    nc.scalar.activation(ptA_v[:, :, q0:], stA_v[:, :, q0:],
                                         Exp, scale=0.125)
                    nc.scalar.activation(ptB[:, q0:], stB[:, q0:], Exp,
                                         scale=0.125)
                    if m >= 8 * qg:
                        # multiplicative 0/1 causal mask on the probabilities
                        w = m - 8 * qg
                        span = P * (w // 2 + 1)
                        if span > q0:
                            for h in range(GH):
                                dst = (ptA[:, h * QGS + q0:h * QGS + span]
                                       if h < 2 else ptB[:, q0:span])
                                nc.gpsimd.tensor_tensor(
                                    dst, dst,
                                    mask_sb[:, w * QGS + q0:w * QGS + span],
                                    mult)
                    for h in range(GH):
                        src = (ptA[:, h * QGS + q0:(h + 1) * QGS] if h < 2
                               else ptB[:, q0:])
                        nc.tensor.matmul(
                            pv[h][:, q0:], v_sb[:, m, h, :], src,
                            start=(m == 0), stop=(m == kcnt - 1))
                # normalize: out_h = pv_h[0:64] / pv_h[64] ; stack for out-proj
                stk0 = stkpool.tile([P, QGS], bf16, tag="sc0")
                stk1 = stkpool.tile([65, QGS], bf16, tag="sc1")
                nc.vector.memset(stk1[64:65, :], 1.0)
                for h in range(GH):
                    recip = nrmpool.tile([1, QGS], bf16, tag="recip")
                    with nc.allow_low_precision(
                            reason="bf16 softmax denominators, ~0.4% rel"):
                        nc.vector.reciprocal(recip[:], pv[h][64:65, :])
                    bcast = stBps.tile([64, QGS], f32, tag="stB",
                                       name=f"bc{qg}_{h}")
                    nc.tensor.matmul(
                        bcast[:], ones_sb[:], recip[:], start=True, stop=True)
                    tmp = nrmpool.tile([64, QGS], bf16, tag="tmp")
                    nc.vector.tensor_copy(tmp[:], pv[h][0:64, :])
                    tgt = stk0[h * 64:(h + 1) * 64, :] if h < 2 else stk1[0:64, :]
                    nc.vector.tensor_tensor(tgt, tmp[:], bcast[:], mult)
                # output projection for this group's 4 query tiles
                for jj in range(QG):
                    op = stAps.tile([P, 1024], f32, tag="stA",
                                    name=f"op{qg}_{jj}")
                    for half in range(2):
                        nc.tensor.matmul(
                            op[:, half * 512:half * 512 + 384],
                            stk0[:, jj * P:(jj + 1) * P],
                            wota0_sb[:, half * 384:(half + 1) * 384],
                            start=True, stop=False)
                        nc.tensor.matmul(
                            op[:, half * 512:half * 512 + 384],
                            stk1[:, jj * P:(jj + 1) * P],
                            wota1_sb[:, half * 384:(half + 1) * 384],
                            start=False, stop=True)
                    oe = oevpool.tile([P, D], f32, tag="oe")
                    opv = op[:].rearrange("p (t x) -> p t x", x=512)
                    nc.vector.tensor_copy(
                        oe[:].rearrange("p (t x) -> p t x", x=384),
                        opv[:, :, 0:384])
                    jq = 4 * qg + jj
                    nc.sync.dma_start(out[jq * P:(jq + 1) * P, :], oe[:])

    nc.compile()
    return nc


def _host_prep(inputs, Wq, bq, Wk, bk, Wv, bv, Wo, bo):
    import ml_dtypes

    bf16 = ml_dtypes.bfloat16
    f8 = ml_dtypes.float8_e4m3
    X = np.asarray(inputs, np.float32).reshape(S, D)
    XT = np.ascontiguousarray(X.T)                      # [768, 4096]
    XT_bf = XT.astype(bf16)
    XT_f8 = XT.astype(f8)
    # query-set gathers: blocks s, s+2, ... of 32 128-col blocks
    XTb = XT.reshape(D, NKB // 2, 2, P)
    XTq = [np.ascontiguousarray(XTb[:, :, s, :].reshape(D, SL)) for s in range(2)]

    # per-core multiplicative causal masks [128, 8*512], 1=keep 0=drop
    # (S_T layout: k on partitions, q on free dim)
    tri = (np.arange(P)[None, :] >= np.arange(P)[:, None]).astype(np.float32)
    mk = []
    for s_ in range(2):
        m = np.ones((P, 8, QGS), np.float32)
        for w in range(8):
            npref = max(0, -(-(w - s_) // 2))  # ceil((w - s)/2) clamped at 0
            m[:, w, :P * npref] = 0.0
            if w >= s_ and (w - s_) % 2 == 0:
                dblk = (w - s_) // 2
                m[:, w, dblk * P:(dblk + 1) * P] = tri
        mk.append(np.ascontiguousarray(m.reshape(P, 8 * QGS)).astype(bf16))

    zeros8 = np.zeros((P, 2 * SL), f8)

    def dual_rail(WT):  # [768, 192] f32 -> [128, 6, 2, 192] fp8 (hi, lo)
        Ws = WT * WSCALE
        hi = Ws.astype(f8)
        lo = (Ws - hi.astype(np.float32)).astype(f8)
        w2 = np.zeros((P, NC, 2, GD), np.float32)
        for c in range(NC):
            w2[:, c, 0, :] = hi[c * P:(c + 1) * P].astype(np.float32)
            w2[:, c, 1, :] = lo[c * P:(c + 1) * P].astype(np.float32)
        return np.ascontiguousarray(w2).astype(f8)

    in_maps = []
    for g in range(NG):
        hs = slice(GD * g, GD * (g + 1))
        WqT = np.ascontiguousarray(Wq[hs, :].T).astype(np.float32)
        WkT = np.ascontiguousarray(Wk[hs, :].T).astype(np.float32)
        WvT = np.ascontiguousarray(Wv[hs, :].T).astype(bf16)
        WoT = np.ascontiguousarray(Wo[:, hs].T).astype(np.float32)  # [192, 768]
        bo_g = bv[hs].astype(np.float32) @ WoT
        if g == 0:
            bo_g = bo_g + bo.astype(np.float32)
        wota = np.concatenate([WoT, bo_g[None, :]], axis=0)  # [193, 768]
        wota0 = np.ascontiguousarray(wota[0:P]).astype(bf16)
        wota1 = np.ascontiguousarray(wota[P:]).astype(bf16)
        bias_t = np.zeros((P, 4), np.float32)
        bias_t[:, 0] = bq[hs][0:128]
        bias_t[0:64, 1] = bq[hs][128:192]
        bias_t[:, 2] = bk[hs][0:128]
        bias_t[0:64, 3] = bk[hs][128:192]
        base = {
            "xt": XT_bf, "wvt": WvT,
            "wota0": wota0, "wota1": wota1,
            "bias": bias_t, "zeros8": zeros8,
        }
        if DRPROJ:
            base["xt8"] = XT_f8
            base["wq2"] = dual_rail(WqT)
            base["wk2"] = dual_rail(WkT)
        else:
            base["wqt"] = WqT.astype(bf16)
            base["wkt"] = WkT.astype(bf16)
        for s_ in range(2):
            m = dict(base)
            m["masks"] = mk[s_]
            if DRPROJ:
                m["xtq8"] = XTq[s_].astype(f8)
            else:
                m["xtq"] = XTq[s_].astype(bf16)
            in_maps.append(m)
    return in_maps


def _gather(results):
    out = np.zeros((S, D), np.float32)
    ov = out.reshape(NQB, 2, P, D)
    for s_ in range(2):
        acc = np.zeros((SL, D), np.float32)
        for g in range(NG):
            acc += np.asarray(results[2 * g + s_]["out"], np.float32)
        ov[:, s_, :, :] = acc.reshape(NQB, P, D)
    return out.reshape(1, S, D)


def kernel(inputs, Wq, bq, Wk, bk, Wv, bv, Wo, bo):
    from concourse.bass_utils import run_bass_kernel_spmd

    if "nc" not in _CACHE:
        _CACHE["nc"] = _build_program()
    nc = _CACHE["nc"]
    in_maps = _host_prep(
        np.asarray(inputs), np.asarray(Wq), np.asarray(bq), np.asarray(Wk),
        np.asarray(bk), np.asarray(Wv), np.asarray(bv), np.asarray(Wo),
        np.asarray(bo))
    # core order: core = 2*g + s; in_maps was built g-major with s inner.
    res = run_bass_kernel_spmd(nc, in_maps, list(range(8))).results
    return _gather(res)


# revision 5
# speedup vs baseline: 1.1433x; 1.0888x over previous
"""Multi-head causal self-attention (D=768, H=12, S=4096) on 8 Trainium2 cores.

Sharding: 4 head-groups (3 heads each) x 2 interleaved query-sets.
Core c = 2*g + s owns head-group g (heads 3g..3g+2) and query 128-row
blocks s, s+2, s+4, ... (even/odd interleave balances the causal
triangle).  Every core runs the SAME program; per-core behaviour is
driven entirely by input data.  Each core produces a partial [2048, 768]
output (its heads pushed through its slice of Wo, all biases folded
in); the host sums the 4 group partials per query-set and re-interleaves
rows.

v3 key points:
  - QK^T matmuls are fp8e4 DoubleRow (half the PE columns): Q and K live
    in zero-padded [128, 2, *] fp8 tiles so all three heads share one
    stationary operand; zero regions kill cross-head terms.
  - Q/K projections are fp8 DoubleRow with dual-rail (hi+lo) fp8 weights.
  - attention runs on 256-query tiles with all 3 heads' scores packed in
    ONE psum tile (2 banks, so double-buffered tiles stay bank-disjoint:
    PE-write + ACT-read on the same bank is fatal), giving a single exp
    instr per key block and a psum budget of 4 (scores) + 3 (pv) +
    1 (projection) banks.  Projections round-robin the one spare bank and
    are emitted one chunk ahead so they hide under attention.
  - normalize uses gpsimd partition_broadcast + a fused psum*bf16 DVE
    multiply; causal-mask multiplies run on GPSIMD; V/P/O stay bf16
    (fp8 there costs ~2.4% output noise).
"""

import numpy as np

D = 768
S = 4096
H = 12
HD = 64
NG = 4          # head groups
GH = 3          # heads per group
GD = GH * HD    # 192 dims per group
SL = S // 2     # local queries per core (2048)
P = 128
NC = D // P     # 6 contraction chunks
QG = 4          # projection chunks (512 q each)
QGS = 512
NT = 8          # attention query groups (256 q each)
QT = 256
NKB = S // P    # 32 key blocks
NQB = SL // P   # 16 local query tiles

DRPROJ = True   # Q/K projections via fp8 dual-rail DoubleRow
WSCALE = 16.0   # pow-2 scale keeping dual-rail fp8 weights in normal range

_CACHE = {}


def _build_program():
    import concourse.bacc as bacc
    import concourse.mybir as mybir
    import concourse.tile as tile
    from contextlib import ExitStack

    bf16 = mybir.dt.bfloat16
    f8 = mybir.dt.float8e4
    f32 = mybir.dt.float32

    nc = bacc.Bacc("TRN2", target_bir_lowering=False, debug=False, num_devices=8)

    xt = nc.dram_tensor("xt", [D, S], bf16, kind="ExternalInput").ap()
    bias = nc.dram_tensor("bias", [P, 4], f32, kind="ExternalInput").ap()
    masks = nc.dram_tensor("masks", [P, 4 * QT], bf16, kind="ExternalInput").ap()
    wvt = nc.dram_tensor("wvt", [D, GD], bf16, kind="ExternalInput").ap()
    wota0 = nc.dram_tensor("wota0", [P, D], bf16, kind="ExternalInput").ap()
    wota1 = nc.dram_tensor("wota1", [65, D], bf16, kind="ExternalInput").ap()
    zeros8 = nc.dram_tensor("zeros8", [P, 2 * SL], f8, kind="ExternalInput").ap()
    out = nc.dram_tensor("out", [SL, D], f32, kind="ExternalOutput").ap()
    if DRPROJ:
        xt8 = nc.dram_tensor("xt8", [D, S], f8, kind="ExternalInput").ap()
        xtq8 = nc.dram_tensor("xtq8", [D, SL], f8, kind="ExternalInput").ap()
        wq2 = nc.dram_tensor("wq2", [P, NC, 2, GD], f8, kind="ExternalInput").ap()
        wk2 = nc.dram_tensor("wk2", [P, NC, 2, GD], f8, kind="ExternalInput").ap()
    else:
        xtq = nc.dram_tensor("xtq", [D, SL], bf16, kind="ExternalInput").ap()
        wqt = nc.dram_tensor("wqt", [D, GD], bf16, kind="ExternalInput").ap()
        wkt = nc.dram_tensor("wkt", [D, GD], bf16, kind="ExternalInput").ap()

    Exp = mybir.ActivationFunctionType.Exp
    mult = mybir.AluOpType.mult
    add = mybir.AluOpType.add
    DR = mybir.MatmulPerfMode.DoubleRow

    with tile.TileContext(nc) as tc, ExitStack() as ctx:
        const = ctx.enter_context(tc.tile_pool(name="const", bufs=1))

        # ---- persistent SBUF tiles ----
        xt_sb = const.tile([P, NC, S], bf16, tag="xt")
        wvt_sb = const.tile([P, NC, GD], bf16, tag="wvt")
        wota0_sb = const.tile([P, D], bf16, tag="wota0")
        wota1_sb = const.tile([65, D], bf16, tag="wota1")
        bias_sb = const.tile([P, 4], f32, tag="bias")
        mask_sb = const.tile([P, 4, QT], bf16, tag="masks")
        if DRPROJ:
            xt8_sb = const.tile([P, NC, S], f8, tag="xt8")
            xtq8_sb = const.tile([P, NC, SL], f8, tag="xtq8")
            wq2_sb = const.tile([P, NC, 2, GD], f8, tag="wq2")
            wk2_sb = const.tile([P, NC, 2, GD], f8, tag="wk2")
        else:
            xtq_sb = const.tile([P, NC, SL], bf16, tag="xtq")
            wqt_sb = const.tile([P, NC, GD], bf16, tag="wqt")
            wkt_sb = const.tile([P, NC, GD], bf16, tag="wkt")
        # fp8 K cache, 3 heads packed into (partition-half, rail):
        #   (p 0:64,  rail 0) = head0 dims, (p 64:128, rail 0) = head1 dims,
        #   (p 0:64,  rail 1) = head2 dims, (p 64:128, rail 1) = ZERO.
        k8_sb = const.tile([P, 2, S], f8, tag="k8")
        # per-head fp8 Q, same packing; zeros everywhere except own region so
        # the shared K stationary operand only contracts the own head's dims.
        q8_sb = [const.tile([P, 2, SL], f8, tag=f"q8{h}", name=f"q8{h}")
                 for h in range(GH)]
        # V per key block: [128 k-part, kb, head, 65] with col 64 = 1.0
        v_sb = const.tile([P, NKB, GH, 65], bf16, tag="vall")

        # ---- DMAs: weights/zeros on the gpsimd (SWDGE) queue ----
        if DRPROJ:
            nc.gpsimd.dma_start(wq2_sb[:], wq2[:])
            nc.gpsimd.dma_start(wk2_sb[:], wk2[:])
        nc.gpsimd.dma_start(bias_sb[:], bias[:])
        for h in range(GH):
            nc.gpsimd.dma_start(q8_sb[h][:], zeros8.rearrange("p (a b) -> p a b", a=2))
        nc.gpsimd.dma_start(k8_sb[64:128, 1, :], zeros8[0:64, :])
        nc.gpsimd.dma_start(mask_sb[:], masks.rearrange("p (w x) -> p w x", x=QT))
        nc.gpsimd.dma_start(wvt_sb[:], wvt.rearrange("(c p) d -> p c d", p=P))
        nc.gpsimd.dma_start(wota0_sb[:], wota0[:])
        nc.gpsimd.dma_start(wota1_sb[:], wota1[:])

        # ---- input windows, JIT-ordered per 512-col group (SP queue) ----
        xt_r = xt.rearrange("(c p) s -> p c s", p=P)
        if DRPROJ:
            xt8_r = xt8.rearrange("(c p) s -> p c s", p=P)
            xtq8_r = xtq8.rearrange("(c p) s -> p c s", p=P)
        else:
            xtq_r = xtq.rearrange("(c p) s -> p c s", p=P)
        for qg in range(QG):
            qw = slice(qg * QGS, (qg + 1) * QGS)
            if DRPROJ:
                nc.sync.dma_start(xtq8_sb[:, :, qw], xtq8_r[:, :, qw])
            else:
                nc.sync.dma_start(xtq_sb[:, :, qw], xtq_r[:, :, qw])
            for kg in (2 * qg, 2 * qg + 1):
                kw = slice(kg * QGS, (kg + 1) * QGS)
                if DRPROJ:
                    nc.sync.dma_start(xt8_sb[:, :, kw], xt8_r[:, :, kw])
                nc.sync.dma_start(xt_sb[:, :, kw], xt_r[:, :, kw])

        nc.vector.memset(v_sb[:, :, :, 64:65], 1.0)

        evsc = 1.0 / WSCALE if DRPROJ else 1.0

        def proj_mms(ps, w2_sb, wt_sb, x8_sb, x_sb, cols, mrange):
            """contraction loop for one Q/K projection psum"""
            n = cols.stop - cols.start
            for c in range(NC):
                if DRPROJ:
                    rhs = (x8_sb[:, c, cols].unsqueeze(1)
                           .broadcast_to([P, 2, n]))
                    nc.tensor.matmul(ps[:], w2_sb[:, c, :, mrange], rhs,
                                     start=(c == 0), stop=(c == NC - 1),
                                     perf_mode=DR)
                else:
                    nc.tensor.matmul(ps[:], wt_sb[:, c, mrange],
                                     x_sb[:, c, cols],
                                     start=(c == 0), stop=(c == NC - 1))

        with tc.tile_pool(name="sc_ps", bufs=2, space="PSUM") as scps, \
             tc.tile_pool(name="pv_ps", bufs=3, space="PSUM") as pvps, \
             tc.tile_pool(name="pa_ps", bufs=1, space="PSUM") as paps, \
             tc.tile_pool(name="pt", bufs=8) as ptpool, \
             tc.tile_pool(name="stk", bufs=2) as stkpool, \
             tc.tile_pool(name="nrm", bufs=4) as nrmpool, \
             tc.tile_pool(name="oev", bufs=4) as oevpool:

            def qproj(qg, use_sc):
                qw = slice(qg * QGS, (qg + 1) * QGS)
                if use_sc:
                    ps = scps.tile([P, 2 * QGS], f32, tag="sc",
                                   name=f"qps{qg}")[:, 0:QGS]
                else:
                    ps = paps.tile([P, QGS], f32, tag="pa", name=f"qps{qg}")[:]
                proj_mms(ps, DRPROJ and wq2_sb, not DRPROJ and wqt_sb,
                         DRPROJ and xtq8_sb, not DRPROJ and xtq_sb,
                         qw, slice(0, 128))
                nc.vector.tensor_scalar(
                    q8_sb[0][0:64, 0, qw], ps[0:64, :],
                    evsc, bias_sb[0:64, 0:1], mult, add)
                nc.vector.tensor_scalar(
                    q8_sb[1][64:128, 0, qw], ps[64:128, :],
                    evsc, bias_sb[64:128, 0:1], mult, add)
                ps2 = paps.tile([64, QGS], f32, tag="pa", name=f"qps2_{qg}")
                proj_mms(ps2[:], DRPROJ and wq2_sb, not DRPROJ and wqt_sb,
                         DRPROJ and xtq8_sb, not DRPROJ and xtq_sb,
                         qw, slice(128, 192))
                nc.vector.tensor_scalar(
                    q8_sb[2][0:64, 1, qw], ps2[:],
                    evsc, bias_sb[0:64, 1:2], mult, add)

            def kproj(kg, use_sc):
                kcols = slice(kg * QGS, (kg + 1) * QGS)
                if use_sc:
                    ps = scps.tile([P, 2 * QGS], f32, tag="sc",
                                   name=f"kps{kg}")[:, 0:QGS]
                else:
                    ps = paps.tile([P, QGS], f32, tag="pa", name=f"kps{kg}")[:]
                proj_mms(ps, DRPROJ and wk2_sb, not DRPROJ and wkt_sb,
                         DRPROJ and xt8_sb, not DRPROJ and xt_sb,
                         kcols, slice(0, 128))
                nc.vector.tensor_scalar(
                    k8_sb[:, 0, kcols], ps[:],
                    evsc, bias_sb[:, 2:3], mult, add)
                ps2 = paps.tile([64, QGS], f32, tag="pa", name=f"kps2_{kg}")
                proj_mms(ps2[:], DRPROJ and wk2_sb, not DRPROJ and wkt_sb,
                         DRPROJ and xt8_sb, not DRPROJ and xt_sb,
                         kcols, slice(128, 192))
                nc.vector.tensor_scalar(
                    k8_sb[0:64, 1, kcols], ps2[:],
                    evsc, bias_sb[0:64, 3:4], mult, add)

            def vproj(kb):
                psv = paps.tile([P, GD], f32, tag="pa", name=f"vps{kb}")
                for c in range(NC):
                    nc.tensor.matmul(
                        psv[:], xt_sb[:, c, kb * P:(kb + 1) * P],
                        wvt_sb[:, c, :],
                        start=(c == 0), stop=(c == NC - 1))
                nc.vector.tensor_copy(
                    v_sb[:, kb, :, 0:64],
                    psv[:].rearrange("p (h d) -> p h d", d=HD))

            def proj_partA(qg):
                use_sc = qg == 0
                qproj(qg, use_sc)
                kproj(2 * qg, use_sc)
                for kb in range(8 * qg, 8 * qg + 4):
                    vproj(kb)

            def proj_partB(qg):
                kproj(2 * qg + 1, False)
                for kb in range(8 * qg + 4, 8 * qg + 8):
                    vproj(kb)

            def attention(t):
                kcnt = 4 * (t + 1)
                pv = [pvps.tile([65, QGS], f32, tag="pv", name=f"pv{t}_{h}")
                      for h in range(GH)]
                for m in range(kcnt):
                    # causal query-suffix trim: for key block m, local query
                    # tiles j < ceil((m-1)/2) - 2t are entirely below the
                    # diagonal for BOTH parities; data masks handle leftovers.
                    q0 = P * (max(0, -(-(m - 1) // 2) - 2 * t) if m > 0 else 0)
                    sc = scps.tile([P, 2 * QGS], f32, tag="sc",
                                   name=f"sc{t}_{m}")
                    sc_v = sc[:].rearrange("p (h x) -> p h x", x=QT)
                    kblk = k8_sb[:, :, m * P:(m + 1) * P]
                    for h in range(GH):
                        nc.tensor.matmul(
                            sc_v[:, h, q0:], kblk,
                            q8_sb[h][:, :, t * QT + q0:(t + 1) * QT],
                            start=True, stop=True, perf_mode=DR)
                    pt = ptpool.tile([P, GH, QT], bf16, tag="pt")
                    # scores are raw q.k; the 1/sqrt(64) lives in the Exp scale
                    nc.scalar.activation(pt[:, :, q0:], sc_v[:, 0:GH, q0:],
                                         Exp, scale=0.125)
                    if m >= 4 * t:
                        # multiplicative 0/1 causal mask on the probabilities
                        w = m - 4 * t
                        span = P * (w // 2 + 1)
                        if span > q0:
                            for h in range(GH):
                                nc.gpsimd.tensor_tensor(
                                    pt[:, h, q0:span], pt[:, h, q0:span],
                                    mask_sb[:, w, q0:span], mult)
                    for h in range(GH):
                        nc.tensor.matmul(
                            pv[h][:, q0:QT], v_sb[:, m, h, :], pt[:, h, q0:],
                            start=(m == 0), stop=(m == kcnt - 1))
                # normalize: out_h = pv_h[0:64] / pv_h[64] ; stack for out-proj
                stk0 = stkpool.tile([P, QT], bf16, tag="sc0")
                stk1 = stkpool.tile([65, QT], bf16, tag="sc1")
                nc.vector.memset(stk1[64:65, :], 1.0)
                for h in range(GH):
                    recip = nrmpool.tile([1, QT], bf16, tag="recip")
                    with nc.allow_low_precision(
                            reason="bf16 softmax denominators, ~0.4% rel"):
                        nc.vector.reciprocal(recip[:], pv[h][64:65, 0:QT])
                    bcast = nrmpool.tile([64, QT], bf16, tag="bcast")
                    nc.gpsimd.partition_broadcast(bcast[:], recip[:])
                    tgt = stk0[h * 64:(h + 1) * 64, :] if h < 2 else stk1[0:64, :]
                    nc.vector.tensor_tensor(tgt, pv[h][0:64, 0:QT], bcast[:],
                                            mult)
                # output projection for this group's 2 query tiles
                for jj in range(2):
                    op0 = paps.tile([P, 384], f32, tag="pa", name=f"op0_{t}_{jj}")
                    op1 = paps.tile([P, 384], f32, tag="pa", name=f"op1_{t}_{jj}")
                    for half, op in ((0, op0), (1, op1)):
                        nc.tensor.matmul(
                            op[:], stk0[:, jj * P:(jj + 1) * P],
                            wota0_sb[:, half * 384:(half + 1) * 384],
                            start=True, stop=False)
                        nc.tensor.matmul(
                            op[:], stk1[:, jj * P:(jj + 1) * P],
                            wota1_sb[:, half * 384:(half + 1) * 384],
                            start=False, stop=True)
                    oe = oevpool.tile([P, D], f32, tag="oe")
                    nc.vector.tensor_copy(oe[:, 0:384], op0[:])
                    nc.vector.tensor_copy(oe[:, 384:768], op1[:])
                    jq = 2 * t + jj
                    nc.sync.dma_start(out[jq * P:(jq + 1) * P, :], oe[:])

            # software-pipelined emission: projections one chunk ahead
            proj_partA(0)
            proj_partB(0)
            proj_partA(1)
            proj_partB(1)
            attention(0)
            attention(1)
            proj_partA(2)
            proj_partB(2)
            attention(2)
            attention(3)
            proj_partA(3)
            proj_partB(3)
            attention(4)
            attention(5)
            attention(6)
            attention(7)

    nc.compile()
    return nc


def _host_prep(inputs, Wq, bq, Wk, bk, Wv, bv, Wo, bo):
    import ml_dtypes

    bf16 = ml_dtypes.bfloat16
    f8 = ml_dtypes.float8_e4m3
    X = np.asarray(inputs, np.float32).reshape(S, D)
    XT = np.ascontiguousarray(X.T)                      # [768, 4096]
    XT_bf = XT.astype(bf16)
    XT_f8 = XT.astype(f8)
    # query-set gathers: blocks s, s+2, ... of 32 128-col blocks
    XTb = XT.reshape(D, NKB // 2, 2, P)
    XTq = [np.ascontiguousarray(XTb[:, :, s, :].reshape(D, SL)) for s in range(2)]

    # per-core multiplicative causal masks [128, 4, 256], 1=keep 0=drop
    # (S_T layout: k on partitions, q on free dim); w = m - 4t
    tri = (np.arange(P)[None, :] >= np.arange(P)[:, None]).astype(np.float32)
    mk = []
    for s_ in range(2):
        m = np.ones((P, 4, QT), np.float32)
        for w in range(4):
            npref = max(0, -(-(w - s_) // 2))  # ceil((w - s)/2) clamped at 0
            m[:, w, :P * npref] = 0.0
            if w >= s_ and (w - s_) % 2 == 0:
                dblk = (w - s_) // 2
                m[:, w, dblk * P:(dblk + 1) * P] = tri
        mk.append(np.ascontiguousarray(m.reshape(P, 4 * QT)).astype(bf16))

    zeros8 = np.zeros((P, 2 * SL), f8)

    def dual_rail(WT):  # [768, 192] f32 -> [128, 6, 2, 192] fp8 (hi, lo)
        Ws = WT * WSCALE
        hi = Ws.astype(f8)
        lo = (Ws - hi.astype(np.float32)).astype(f8)
        w2 = np.zeros((P, NC, 2, GD), np.float32)
        for c in range(NC):
            w2[:, c, 0, :] = hi[c * P:(c + 1) * P].astype(np.float32)
            w2[:, c, 1, :] = lo[c * P:(c + 1) * P].astype(np.float32)
        return np.ascontiguousarray(w2).astype(f8)

    in_maps = []
    for g in range(NG):
        hs = slice(GD * g, GD * (g + 1))
        WqT = np.ascontiguousarray(Wq[hs, :].T).astype(np.float32)
        WkT = np.ascontiguousarray(Wk[hs, :].T).astype(np.float32)
        WvT = np.ascontiguousarray(Wv[hs, :].T).astype(bf16)
        WoT = np.ascontiguousarray(Wo[:, hs].T).astype(np.float32)  # [192, 768]
        bo_g = bv[hs].astype(np.float32) @ WoT
        if g == 0:
            bo_g = bo_g + bo.astype(np.float32)
        wota = np.concatenate([WoT, bo_g[None, :]], axis=0)  # [193, 768]
        wota0 = np.ascontiguousarray(wota[0:P]).astype(bf16)
        wota1 = np.ascontiguousarray(wota[P:]).astype(bf16)
        bias_t = np.zeros((P, 4), np.float32)
        bias_t[:, 0] = bq[hs][0:128]
        bias_t[0:64, 1] = bq[hs][128:192]
        bias_t[:, 2] = bk[hs][0:128]
        bias_t[0:64, 3] = bk[hs][128:192]
        base = {
            "xt": XT_bf, "wvt": WvT,
            "wota0": wota0, "wota1": wota1,
            "bias": bias_t, "zeros8": zeros8,
        }
        if DRPROJ:
            base["xt8"] = XT_f8
            base["wq2"] = dual_rail(WqT)
            base["wk2"] = dual_rail(WkT)
        else:
            base["wqt"] = WqT.astype(bf16)
            base["wkt"] = WkT.astype(bf16)
        for s_ in range(2):
            m = dict(base)
            m["masks"] = mk[s_]
            if DRPROJ:
                m["xtq8"] = XTq[s_].astype(f8)
            else:
                m["xtq"] = XTq[s_].astype(bf16)
            in_maps.append(m)
    return in_maps


def _gather(results):
    out = np.zeros((S, D), np.float32)
    ov = out.reshape(NQB, 2, P, D)
    for s_ in range(2):
        acc = np.zeros((SL, D), np.float32)
        for g in range(NG):
            acc += np.asarray(results[2 * g + s_]["out"], np.float32)
        ov[:, s_, :, :] = acc.reshape(NQB, P, D)
    return out.reshape(1, S, D)


def kernel(inputs, Wq, bq, Wk, bk, Wv, bv, Wo, bo):
    from concourse.bass_utils import run_bass_kernel_spmd

    if "nc" not in _CACHE:
        _CACHE["nc"] = _build_program()
    nc = _CACHE["nc"]
    in_maps = _host_prep(
        np.asarray(inputs), np.asarray(Wq), np.asarray(bq), np.asarray(Wk),
        np.asarray(bk), np.asarray(Wv), np.asarray(bv), np.asarray(Wo),
        np.asarray(bo))
    # core order: core = 2*g + s; in_maps was built g-major with s inner.
    res = run_bass_kernel_spmd(nc, in_maps, list(range(8))).results
    return _gather(res)
